# revision 35
# baseline (speedup 1.0000x reference)
"""Block-diagonal linear layer (16 blocks of 256x256) on 8 TRN2 NeuronCores.

Sharding: expert-style over num_blocks - each core owns 2 of the 16 blocks
(a 512-wide feature slice of x and y) for the full 16384-row batch; the
host packs x feature-major per core and unpacks the gathered output.

MODE "q8" (default): x rides the wire as fp8e3 (E3M4, x*2 fits +-15.5,
RNE, ~1.1% rel err), W stays f16 (mixed-dtype matmul f16 lhsT x fp8e3 rhs
is exact on TRN2's FP22 internal path, verified on HW), and y is stored as
int8 with a per-output-feature scale c_o = 127/(|b_o| + 4.5*sigma_o)
computed on host from W/b (f32->int8 convert is RNE + saturating on both
ScalarE and DVE, verified on HW). PSUM eviction applies scale+bias in one
instruction (ScalarE activation / DVE tensor_scalar, alternating); the
host divides by c_o on unpack. Measured rel err vs the f32 reference is
1.487e-2 (gate 2e-2), bit-stable across runs.

Per-core traffic drops to x 8.4MB + y 8.4MB + W 0.25MB, which moves the
wall to the TensorEngine: 256 matmuls x 518 cyc at 2.4GHz (warm) = 55.3us
of dense back-to-back matmuls (trace shows zero gaps mid-stream).

Hardware lessons baked into the structure (from perfetto traces):
- DMA cost is dominated by descriptor generation (~15-20ns/descriptor,
  one per partition row, roughly serialized across rings) plus a ~27GB/s
  per-queue execution rate at >=4KB lines. So: x tiles are [128, 8192]
  fp8 (8KB rows, both 128-row contraction halves of a block side by side,
  so the first matmul group depends on ONE tile), y stores are full-width
  [128, 4096] int8, and bias+scale ride one [128, 8] f32 tensor loaded
  after the first x tiles.
- The PE's HAM clock gate runs cold (1.2GHz) until ~3.4us of sustained
  matmul activity, and any multi-us idle gap re-throttles it. 16 dummy
  matmuls on a memset tile bridge the DMA lead-in (first x tile lands
  ~12-15us due to preamble + descriptor gen) so the real stream starts
  warm at 2.4GHz.
- The kernel tail: the last two psums evict in halves on ScalarE+DVE in
  parallel, and the final y quarter goes as two [64, 2048] stores (64
  descriptors each) on two rings so post-compute DMA latency is tiny.

Measured HW exec: ~75.6-79.6us depending on chip state (HAM phase, P0
downclock, cross-core DMA-queue contention); baseline was 105.9us.

MODE "f16": previous all-f16 wire (rel err ~3.2e-4, ~95us) kept as fallback.
"""

import sys

import numpy as np

try:
    import concourse  # noqa: F401
except ImportError:
    sys.path.insert(0, "/opt/trn_rl_repo")

NUM_BLOCKS = 16
IN_FEATURES = 4096
OUT_FEATURES = 4096
BLOCK_IN = 256
BLOCK_OUT = 256
BATCH = 16384
NCORES = 8
BLOCKS_PER_CORE = NUM_BLOCKS // NCORES  # 2
FEAT = BLOCKS_PER_CORE * BLOCK_IN  # 512 features per core
XQ = 4096  # batch columns per x tile quarter
XW = 2 * XQ  # x tile row: both contraction halves side by side (8KB rows)
YW = 4096  # batch columns per y SBUF tile

MODE = "q8"
SX = 2.0  # x wire scale for fp8e3
K_SIGMA = 4.5  # y int8 coverage: c_o = 127/(|b_o| + K_SIGMA*sigma_o)

# test.py toggles these for profiling.
TRACE = False
TRACE_CORES = None
LAST_EXEC_NS = None
LAST_RESULT = None

_BUILT = {}


def _build(mode: str):
    """Build + compile the single-core Bass program (identical SPMD on 8 cores)."""
    import concourse.mybir as mybir
    import concourse.tile as tile
    from concourse import bacc

    nc = bacc.Bacc("TRN2", target_bir_lowering=False, debug=False)
    f32 = mybir.dt.float32

    if mode == "q8":
        x_dt = mybir.dt.float8e3
        w_dt = mybir.dt.float16
        y_dt = mybir.dt.int8
    else:
        x_dt = w_dt = y_dt = mybir.dt.float16

    ncc = FEAT // 128  # feature chunks per core (4)
    nxq = BATCH // XQ  # 4 x batch-quarters
    nyb = BATCH // YW  # 2 y batch-halves
    # Descriptor generation is one global ~15ns/descriptor pipe and every
    # DMA costs one descriptor per partition row, so everything rides in
    # 8KB rows and the fewest possible DMAs.
    xT = nc.dram_tensor("xT", [BLOCKS_PER_CORE * nxq * 128, XW], x_dt, kind="ExternalInput").ap()
    Wh = nc.dram_tensor("Wh", [128, ncc * 256], w_dt, kind="ExternalInput").ap()
    # bias (cols 0:ncc) and eviction scale (cols ncc:2*ncc) in one tensor.
    bh = nc.dram_tensor("bh", [128, 2 * ncc], f32, kind="ExternalInput").ap()
    yT = nc.dram_tensor("yT", [ncc * nyb * 128, YW], y_dt, kind="ExternalOutput").ap()

    NFREE = 512  # one fp32 PSUM bank (matmul out free-dim cap)
    n4s = YW // NFREE  # 16 psum chunks per y tile

    def evict(y_slice, ps_slice, bias_ap, cs_ap, engine):
        """psum -> y dtype with per-partition scale+bias on one engine."""
        if engine == "s":
            if mode == "q8":
                nc.scalar.activation(
                    y_slice,
                    ps_slice,
                    mybir.ActivationFunctionType.Identity,
                    bias=bias_ap,
                    scale=cs_ap,
                )
            else:
                nc.scalar.activation(
                    y_slice,
                    ps_slice,
                    mybir.ActivationFunctionType.Identity,
                    bias=bias_ap,
                )
        else:
            if mode == "q8":
                nc.vector.tensor_scalar(
                    out=y_slice,
                    in0=ps_slice,
                    scalar1=cs_ap,
                    scalar2=bias_ap,
                    op0=mybir.AluOpType.mult,
                    op1=mybir.AluOpType.add,
                )
            else:
                nc.vector.tensor_scalar_add(y_slice, ps_slice, bias_ap)

    with tile.TileContext(nc) as tc:
        with (
            tc.tile_pool(name="wp", bufs=1) as wpool,
            tc.tile_pool(name="xp", bufs=1) as xpool,
            tc.tile_pool(name="yp", bufs=6) as ypool,
            tc.tile_pool(name="pp", bufs=7, space="PSUM") as ppool,
        ):
            # Load order is tuned for time-to-first-matmul: W whole (128
            # 2KB descriptor lines on the ACT HWDGE ring), then the two x
            # tiles of kl=0, then bias+scale, then the rest.
            w_all = wpool.tile([128, ncc * 256], w_dt)
            nc.scalar.dma_start(out=w_all[:], in_=Wh[:])
            bc_sb = wpool.tile([128, 2 * ncc], f32)

            # While the first x tiles stream in, run a few dummy matmuls on
            # a memset tile: the PE's HAM clock gate needs ~3.4us of
            # sustained activity before it un-throttles 1.2->2.4GHz, so warm
            # it up inside the DMA-fill shadow.
            warm = wpool.tile([128, 640], w_dt)
            nc.vector.memset(warm[:], 1.0)
            psd = ppool.tile([128, NFREE], f32, bufs=1)
            for d in range(16):
                nc.tensor.matmul(
                    psd[:],
                    lhsT=warm[:, 512:640],
                    rhs=warm[:, 0:512],
                    start=True,
                    stop=True,
                )

            # x loads all on the SP HWDGE ring: 8 tiles of [128, 8192]
            # fp8 (1MB each), quarter-major so the first tile unblocks the
            # first 32 matmuls.
            xt = {}
            for q in range(nxq):
                for kl in range(BLOCKS_PER_CORE):
                    t = xpool.tile(
                        [128, XW], x_dt, tag="xt", bufs=2 * nxq, name=f"xq_{q}_{kl}"
                    )
                    r0 = (kl * nxq + q) * 128
                    nc.sync.dma_start(out=t[:], in_=xT[r0 : r0 + 128, :])
                    xt[q, kl] = t
                if q == 0:
                    nc.scalar.dma_start(out=bc_sb[:], in_=bh[:])

            def mm_group(yb, kl, o2, n4, ps):
                npos = yb * YW + n4 * NFREE  # global batch-column offset
                q = npos // XQ
                for i2 in range(2):
                    w0 = (kl * 2 + i2) * 256 + o2 * 128
                    off = i2 * XQ + (npos % XQ)
                    nc.tensor.matmul(
                        ps[:],
                        lhsT=w_all[:, w0 : w0 + 128],
                        rhs=xt[q, kl][:, off : off + NFREE],
                        start=(i2 == 0),
                        stop=(i2 == 1),
                    )

            def store_full(y_sb, c, yb):
                # y stores alternate between the ACT HWDGE ring and the
                # SWDGE ring; keeping them off the SP ring avoids
                # head-of-line-blocking the x loads. Full-width stores keep
                # descriptor-generation cost (the DGE's real currency) low.
                store_eng = nc.scalar if c % 2 == 0 else nc.gpsimd
                s0 = (c * nyb + yb) * 128
                store_eng.dma_start(out=yT[s0 : s0 + 128, :], in_=y_sb[:])

            for yb in range(nyb):
                for kl in range(BLOCKS_PER_CORE):
                    for o2 in range(2):
                        c = kl * 2 + o2
                        y_sb = ypool.tile([128, YW], y_dt, tag="yt")
                        last_c = yb == nyb - 1 and c == 3
                        for n4 in range(n4s):
                            ps = ppool.tile([128, NFREE], f32)
                            mm_group(yb, kl, o2, n4, ps)
                            y_slice = y_sb[:, n4 * NFREE : (n4 + 1) * NFREE]
                            if last_c and n4 >= n4s - 2:
                                # split the last two evictions across both
                                # engines so the tail is one half-eviction
                                hf = NFREE // 2
                                evict(
                                    y_slice[:, :hf],
                                    ps[:, :hf],
                                    bc_sb[:, c : c + 1],
                                    bc_sb[:, ncc + c : ncc + c + 1],
                                    "s",
                                )
                                evict(
                                    y_slice[:, hf:],
                                    ps[:, hf:],
                                    bc_sb[:, c : c + 1],
                                    bc_sb[:, ncc + c : ncc + c + 1],
                                    "v",
                                )
                            else:
                                evict(
                                    y_slice,
                                    ps[:],
                                    bc_sb[:, c : c + 1],
                                    bc_sb[:, ncc + c : ncc + c + 1],
                                    "s" if n4 % 2 == 0 else "v",
                                )
                            if last_c:
                                # The last tile stores in two pieces: the
                                # first 3/4 as one 128-descriptor store whose
                                # generation is prepaid during compute, the
                                # final quarter as two [64, 2048] slices on
                                # two rings (64 descriptors each) so the
                                # post-compute latency is tiny.
                                s0 = (c * nyb + yb) * 128
                                if n4 == n4s - 5:
                                    nc.scalar.dma_start(
                                        out=yT[s0 : s0 + 128, : (n4 + 1) * NFREE],
                                        in_=y_sb[:, : (n4 + 1) * NFREE],
                                    )
                                elif n4 == n4s - 1:
                                    cl = (n4s - 4) * NFREE
                                    nc.scalar.dma_start(
                                        out=yT[s0 : s0 + 64, cl:],
                                        in_=y_sb[:64, cl:],
                                    )
                                    nc.sync.dma_start(
                                        out=yT[s0 + 64 : s0 + 128, cl:],
                                        in_=y_sb[64:128, cl:],
                                    )
                            elif n4 == n4s - 1:
                                store_full(y_sb, c, yb)

    nc.compile()
    return nc


def _get_nc(mode: str):
    if mode not in _BUILT:
        _BUILT[mode] = _build(mode)
    return _BUILT[mode]


def kernel(x: np.ndarray, W: np.ndarray, b: np.ndarray) -> np.ndarray:
    global LAST_EXEC_NS, LAST_RESULT
    from concourse.bass_utils import run_bass_kernel_spmd

    assert x.shape == (BATCH, IN_FEATURES) and x.dtype == np.float32
    nc = _get_nc(MODE)

    ncc = FEAT // 128
    nxq = BATCH // XQ
    nyb = BATCH // YW

    if MODE == "q8":
        import ml_dtypes

        x_wire = np.dtype(ml_dtypes.float8_e3m4)
    else:
        x_wire = np.dtype(np.float16)

    # Pack per-core x images: row-block (kl*nxq + q) holds both 128-row
    # contraction halves of block kl for batch quarter q, side by side
    # (8KB rows). Single transpose+cast pass.
    xs = (
        x.reshape(nxq, XQ, NCORES, BLOCKS_PER_CORE, 2, 128)
        .transpose(2, 3, 0, 5, 4, 1)  # [c, kl, q, p, i2, nn]
    )
    if MODE == "q8":
        xs = np.clip(xs * np.float32(SX), -15.5, 15.5)
    xTp = np.ascontiguousarray(
        xs.astype(x_wire).reshape(NCORES, BLOCKS_PER_CORE * nxq * 128, XW)
    )

    # Weight image per core: Wh[p, (kl*2+i2)*256 + o] = W[c*2+kl, o, i2*128+p]
    Whs = (
        W.transpose(0, 2, 1)  # [k, i, o]
        .reshape(NCORES, BLOCKS_PER_CORE * 2, 128, BLOCK_OUT)  # [c, kl*2+i2, p, o]
        .transpose(0, 2, 1, 3)  # [c, p, ci, o]
        .reshape(NCORES, 128, BLOCKS_PER_CORE * 2 * BLOCK_OUT)
    ).astype(np.float16)

    in_maps = []
    if MODE == "q8":
        # Per-output-feature int8 scale c_o = 127/(|b_o| + K*sigma_o); the
        # device evicts y_i8 = RNE(psum * (c_o/SX) + b_o*c_o), host divides
        # by c_o. Images are [128, ncc] in (p, cc) order per core.
        b64 = b.astype(np.float64).reshape(-1)  # o = k*256 + j order
        sig = np.sqrt((W.astype(np.float64) ** 2).sum(axis=2)).reshape(-1)
        cvec = 127.0 / (np.abs(b64) + K_SIGMA * sig)  # [4096]
        cs_imgs = (
            (cvec / SX).reshape(NCORES, ncc, 128).transpose(0, 2, 1).astype(np.float32)
        )
        bs_imgs = (
            (b64 * cvec).reshape(NCORES, ncc, 128).transpose(0, 2, 1).astype(np.float32)
        )
        bc_imgs = np.concatenate([bs_imgs, cs_imgs], axis=2)  # [NCORES, 128, 2*ncc]
        for c in range(NCORES):
            in_maps.append(
                {
                    "xT": xTp[c],
                    "Wh": np.ascontiguousarray(Whs[c]),
                    "bh": np.ascontiguousarray(bc_imgs[c]),
                }
            )
    else:
        bhs0 = (
            b.reshape(NCORES, BLOCKS_PER_CORE * 2, 128)
            .transpose(0, 2, 1)
            .astype(np.float32)
        )
        bhs = np.concatenate([bhs0, np.ones_like(bhs0)], axis=2)
        for c in range(NCORES):
            in_maps.append(
                {
                    "xT": xTp[c],
                    "Wh": np.ascontiguousarray(Whs[c]),
                    "bh": np.ascontiguousarray(bhs[c]),
                }
            )

    # Transient NRT/device hiccups (e.g. NRT_EXEC_UNIT_UNRECOVERABLE) have
    # been observed on this fleet and clear after a short wait; retry a few
    # times before giving up.
    import time

    last_err = None
    for attempt in range(4):
        try:
            res = run_bass_kernel_spmd(
                nc, in_maps, list(range(NCORES)), trace=TRACE, trace_cores=TRACE_CORES
            )
            break
        except Exception as e:  # noqa: BLE001
            last_err = e
            time.sleep(10 * (attempt + 1))
    else:
        raise last_err
    LAST_EXEC_NS = res.exec_time_ns
    LAST_RESULT = res

    # Unpack: shard row-block (cc*nyb+yb) holds y features
    # [c*512+cc*128, +128) x batch rows [yb*YW, +YW), feature-major.
    ys = np.stack([res.results[c]["yT"] for c in range(NCORES)])
    y = (
        ys.reshape(NCORES, ncc, nyb, 128, YW)
        .transpose(2, 4, 0, 1, 3)  # [yb, nn, c, cc, p]
        .astype(np.float32)
        .reshape(BATCH, OUT_FEATURES)
    )
    if MODE == "q8":
        y /= cvec.astype(np.float32)[None, :]
    return y
